# revision 24
# baseline (speedup 1.0000x reference)
import os
import sys

sys.path.insert(0, '/opt/trn_rl_repo')
import numpy as np

NCORES = 8
N = 100000
H = 128
GROUPS = 4
GC = H // GROUPS
K = 5
SHARD_N = N // NCORES          # 12500 owned real nodes per core
TILES = 98
SHARD = TILES * 128            # 12544 padded
FULL = SHARD * NCORES          # 100352
WIN = 2 * SHARD                # 25088 rows per gather window (int16-safe)
NWIN = 4
CLS_N = N // NWIN              # 25000 nodes per window class
NTMAX = 8                      # max tiles per chunk (PSUM partial buffer)
CAPR = 56                      # max gather rounds per chunk (SBUF gbuf)
RMS_EPS = 1.1920929e-07

_cache = {}


def _build(schedule, repeat=1):
    from concourse import bacc, mybir, tile

    (chunks, Q, col0w, TR) = schedule
    # chunks: list of [tile indices]; Q: [nchunk, NWIN] rounds per tile;
    # col0w: [nchunk, NWIN] global start column of each (chunk, window)
    # call; TR: total columns.
    f32 = mybir.dt.float32
    bf16 = mybir.dt.bfloat16
    i16 = mybir.dt.int16

    key = ("v2", TR, os.environ.get("KERNEL_ABLATE"),
           os.environ.get("KERNEL_LCOPY", "1"),
           tuple(Q.ravel()), tuple(tuple(ts) for ts in chunks), repeat)
    if key in _cache:
        return _cache[key]

    ablate = os.environ.get("KERNEL_ABLATE")
    nc = bacc.Bacc("TRN2", target_bir_lowering=False, debug=False,
                   num_devices=NCORES)
    xfb = nc.dram_tensor("x_fullf", [FULL, H], f32, kind="ExternalInput")
    xo = nc.dram_tensor("x_own", [SHARD, H], f32, kind="ExternalInput")
    idx_in = nc.dram_tensor("idx_in", [128, 8 * TR], i16, kind="ExternalInput")
    ew_in = nc.dram_tensor("ew_in", [128, TR], f32, kind="ExternalInput")
    par_in = nc.dram_tensor("par_in", [128, 40], f32, kind="ExternalInput")
    rw_in = nc.dram_tensor("rw_in", [128, H], f32, kind="ExternalInput")
    out_ext = nc.dram_tensor("out", [SHARD, H], f32, kind="ExternalOutput")

    nch = len(chunks)
    t0s = [ts[0] for ts in chunks]

    with tile.TileContext(nc) as tc:
        with (
            tc.tile_pool(name="dram", bufs=1, space="DRAM") as dram,
            tc.tile_pool(name="big", bufs=1) as big,
            tc.tile_pool(name="stage", bufs=2) as stage,
            tc.tile_pool(name="psum", bufs=1, space="PSUM") as psum,
            tc.tile_pool(name="small", bufs=1) as small,
        ):
            idx_sb = big.tile([128, 8 * TR], i16, tag="idx")
            ew_sb = big.tile([128, TR], f32, tag="ew")
            par_sb = big.tile([128, 40], f32, tag="par")
            rw_sb = big.tile([128, H], f32, tag="rw")
            nc.sync.dma_start(idx_sb[:], idx_in[:])
            nc.sync.dma_start(ew_sb[:], ew_in[:])
            nc.sync.dma_start(par_sb[:], par_in[:])
            nc.sync.dma_start(rw_sb[:], rw_in[:])

            TA = big.tile([128, SHARD], f32, tag="TA")
            TB = big.tile([128, SHARD], f32, tag="TB")
            res = big.tile([128, SHARD], bf16, tag="res")
            propb = big.tile([128, NTMAX * H], f32, tag="prop")
            pbuf = psum.tile([128, NWIN, NTMAX, H], f32, tag="pb")

            def sl(ap, t0, t1):  # [128, SHARD] -> [128, (t1-t0)*H] slice
                return ap[:, t0 * H:t1 * H]

            def cheb(k):  # [128, 4] broadcast-ready column block for hop k
                return par_sb[:, 4 * k:4 * k + 4]

            def gv(ap2, nt):  # [128, nt*H] -> [128, nt, 4, GC] group view
                return ap2.rearrange("p (t g c) -> p t g c", g=GROUPS, c=GC)

            def cbc(k, nt):  # cheb col k broadcast to [128, nt, 4, GC]
                return cheb(k).rearrange("p g -> p () g ()").to_broadcast(
                    [128, nt, GROUPS, GC])

            for rep in range(repeat):
                sfx = f"_{rep}" if rep else ""
                shard_b = [dram.tile([SHARD, H], f32, tag=f"sh{k}{sfx}",
                                     name=f"sh{k}{sfx}") for k in range(4)]
                full_b = [dram.tile([FULL, H], f32, tag=f"fl{k}{sfx}",
                                    name=f"fl{k}{sfx}", addr_space="Shared")
                          for k in range(4)]
                # T_prev2 = x (fp32, SBUF resident)
                nc.sync.dma_start(
                    TB[:].rearrange("p (t j) -> p t j", j=H),
                    xo[:].rearrange("(t p) j -> p t j", p=128))

                lcopy = os.environ.get("KERNEL_LCOPY", "0") == "1"
                full_l = [dram.tile([FULL, H], f32, tag=f"fL{k}{sfx}",
                                    name=f"fL{k}{sfx}") for k in range(4)]
                for hop in range(1, K + 1):
                    if hop == 1:
                        src_tab = xfb
                    elif lcopy:
                        src_tab = full_l[hop - 2]
                    else:
                        src_tab = full_b[hop - 2]
                    # hop k >= 2 overwrites the buffer holding T_{k-2}
                    cur = TA if hop % 2 == 1 else TB
                    for s in range(nch):
                        ts = chunks[s]
                        nt = len(ts)
                        t0, t1 = ts[0], ts[-1] + 1
                        rtot = int(nt * Q[s].sum())
                        gbuf = stage.tile([128, CAPR, H], f32, tag="g")
                        prop = propb
                        base = 0
                        for w in range(NWIN):
                            qw = int(Q[s, w])
                            if qw == 0:
                                continue
                            rw_ = nt * qw
                            nidx = rw_ * 128
                            cb = int(col0w[s, w])
                            if ablate != "gather":
                                nc.gpsimd.dma_gather(
                                    gbuf[:, base:base + rw_, :],
                                    src_tab[w * WIN:(w + 1) * WIN, :],
                                    idx_sb[:, 8 * cb:8 * (cb + rw_)],
                                    nidx, nidx, H, single_packet=False,
                                )
                            base += rw_
                        cb0 = int(col0w[s, 0])
                        # bulk ew multiply (bf16, all windows at once)
                        nc.vector.tensor_mul(
                            gbuf[:, :rtot, :], gbuf[:, :rtot, :],
                            ew_sb[:, cb0:cb0 + rtot]
                            .rearrange("p r -> p r ()")
                            .to_broadcast([128, rtot, H]))
                        # per-window segmented reduce over rounds
                        base = 0
                        for w in range(NWIN):
                            qw = int(Q[s, w])
                            if qw == 0:
                                nc.vector.memset(pbuf[:, w, :nt, :], 0.0)
                                continue
                            rw_ = nt * qw
                            if ablate == "dve":
                                base += rw_
                                continue
                            nc.vector.tensor_reduce(
                                pbuf[:, w, :nt, :],
                                gbuf[:, base:base + rw_, :]
                                .rearrange("p (t r) h -> p t h r", r=qw),
                                mybir.AxisListType.X, mybir.AluOpType.add)
                            base += rw_
                        # combine windows
                        ptgt = (sl(cur[:], t0, t1) if hop == 1
                                else prop[:, :nt * H])
                        nc.vector.tensor_reduce(
                            ptgt.rearrange("p (t h) -> p t h", h=H),
                            pbuf[:, :, :nt, :]
                            .rearrange("p w t h -> p t h w"),
                            mybir.AxisListType.X, mybir.AluOpType.add)
                        tmp = pbuf[:, 0, :nt, :].rearrange(
                            "p t h -> p (t h)")
                        if hop >= 2:
                            # T_k = 2*prop - T_{k-2} (in place over T_{k-2})
                            nc.vector.scalar_tensor_tensor(
                                sl(cur[:], t0, t1), prop[:, :nt * H], 2.0,
                                sl(cur[:], t0, t1),
                                mybir.AluOpType.mult,
                                mybir.AluOpType.subtract)
                            # res += c_hop * T_k
                            nc.vector.tensor_mul(
                                gv(tmp, nt), gv(sl(cur[:], t0, t1), nt),
                                cbc(hop, nt))
                            nc.vector.tensor_add(
                                sl(res[:], t0, t1), sl(res[:], t0, t1), tmp)
                        else:
                            # res = c0*x + c1*T1  (TB holds x)
                            nc.vector.tensor_mul(
                                gv(tmp, nt), gv(sl(cur[:], t0, t1), nt),
                                cbc(1, nt))
                            nc.vector.tensor_mul(
                                gv(sl(res[:], t0, t1), nt),
                                gv(sl(TB[:], t0, t1), nt), cbc(0, nt))
                            nc.vector.tensor_add(
                                sl(res[:], t0, t1), sl(res[:], t0, t1), tmp)
                        # store T_k chunk as bf16 (cast DMA) for AllGather
                        if hop <= 4:
                            nc.sync.dma_start(
                                shard_b[hop - 1][t0 * 128:t1 * 128, :]
                                .rearrange("(t p) j -> p t j", p=128),
                                sl(cur[:], t0, t1)
                                .rearrange("p (t j) -> p t j", j=H))
                    if hop <= 4:
                        nc.gpsimd.collective_compute(
                            "AllGather", mybir.AluOpType.bypass,
                            replica_groups=[list(range(NCORES))],
                            ins=[shard_b[hop - 1][:].opt()],
                            outs=[full_b[hop - 1][:].opt()],
                        )
                        if lcopy:
                            # gathers from Shared DRAM are slow; stage the
                            # gathered table into local DRAM first
                            nc.sync.dma_start(full_l[hop - 1][:],
                                              full_b[hop - 1][:])
                # epilogue: res complete; TA/TB are dead scratch now
                def res3(ap):
                    return ap.rearrange("p (t j) -> p t j", j=H)

                gall = res[:].rearrange("p (t g c) -> p t g c", g=GROUPS,
                                        c=GC)

                def pbc(c0):
                    return (par_sb[:, c0:c0 + 4]
                            .rearrange("p g -> p () g ()")
                            .to_broadcast([128, TILES, GROUPS, GC]))

                nc.vector.tensor_mul(gall, gall, pbc(24))
                nc.vector.tensor_add(gall, gall, pbc(28))
                nc.vector.tensor_mul(TB[:], res[:], res[:])
                ssq = small.tile([128, TILES], f32, tag="ssq")
                nc.vector.tensor_reduce(
                    ssq[:], res3(TB[:]), mybir.AxisListType.X,
                    mybir.AluOpType.add)
                rms = small.tile([128, TILES], f32, tag="rms")
                sq = small.tile([128, TILES], f32, tag="sqr")
                nc.scalar.activation(
                    sq[:], ssq[:], mybir.ActivationFunctionType.Sqrt,
                    bias=par_sb[:, 32:33], scale=1.0 / H)
                nc.vector.reciprocal(rms[:], sq[:])
                nc.vector.tensor_mul(
                    res3(res[:]), res3(res[:]),
                    rms[:].rearrange("p (t o) -> p t o", o=1)
                    .to_broadcast([128, TILES, H]))
                nc.vector.tensor_mul(
                    res3(res[:]), res3(res[:]),
                    rw_sb[:].rearrange("p (o j) -> p o j", o=1)
                    .to_broadcast([128, TILES, H]))
                # SiLU = x * sigmoid(x)
                nc.scalar.activation(
                    TB[:], res[:], mybir.ActivationFunctionType.Sigmoid)
                nc.vector.tensor_mul(TA[:], res[:], TB[:])
                nc.sync.dma_start(
                    out_ext[:].rearrange("(t p) j -> p t j", p=128),
                    res3(TA[:]))

    nc.compile()
    _cache[key] = nc
    return nc


def _order_profiles(profs, counts):
    """Greedy nearest-neighbor chain over distinct profile rows (L1)."""
    np_, _ = profs.shape
    visited = np.zeros(np_, bool)
    cur = int(np.argmax(counts))
    order = [cur]
    visited[cur] = True
    for _ in range(np_ - 1):
        d = np.abs(profs - profs[cur]).sum(axis=1).astype(np.float64)
        d[visited] = np.inf
        cur = int(np.argmin(d))
        order.append(cur)
        visited[cur] = True
    return np.array(order)


def _prep(x, edge_weight_norm, edge_index):
    src = np.asarray(edge_index[0]).astype(np.int64)
    dst = np.asarray(edge_index[1]).astype(np.int64)
    ew = np.asarray(edge_weight_norm, dtype=np.float32)
    E = src.shape[0]

    # window class of a src node = v % NWIN; class-w nodes are owned by
    # cores {2w, 2w+1} so their rows fill window w = [w*WIN, (w+1)*WIN).
    prof = np.zeros((N, NWIN), np.int32)
    np.add.at(prof, (dst, src % NWIN), 1)

    # canonical profile order shared by all classes: greedy L1 chain over
    # the distinct profiles of the whole graph
    uniq, uinv, ucnt = np.unique(prof, axis=0, return_inverse=True,
                                 return_counts=True)
    order = _order_profiles(uniq.astype(np.int64), ucnt)
    prank = np.empty(len(uniq), np.int64)
    prank[order] = np.arange(len(uniq))
    node_rank = prank[uinv]  # profile-bucket rank of each node

    perm_pos = np.empty(N, np.int64)
    for w in range(NWIN):
        nodes = np.flatnonzero(np.arange(N) % NWIN == w)
        o = np.argsort(node_rank[nodes], kind='stable')
        sn = nodes[o]
        rank = np.arange(CLS_N)
        core = 2 * w + (rank % 2)
        pos = core * SHARD + rank // 2
        perm_pos[sn] = pos

    def edge_maps(pp):
        src_p = pp[src]
        dst_p = pp[dst]
        dst_core = dst_p // SHARD
        win_id = src_p // WIN
        win_rel = (src_p - win_id * WIN).astype(np.int64)
        dst_local = dst_p - dst_core * SHARD
        tile_id = dst_local // 128
        part_id = dst_local % 128
        cnt = np.zeros((NCORES, TILES, 128, NWIN), np.int32)
        np.add.at(cnt, (dst_core, tile_id, part_id, win_id), 1)
        R_tw = cnt.max(axis=(0, 2)).astype(np.int64)  # [TILES, NWIN]
        return src_p, dst_core, win_id, win_rel, tile_id, part_id, R_tw

    _, _, _, _, _, _, R_tw = edge_maps(perm_pos)

    # cluster tiles into chunks by profile similarity (tiles get relabeled
    # afterwards so each chunk is a contiguous tile range)
    unassigned = set(range(TILES))
    chunks_old = []
    q_list = []
    while unassigned:
        seed = max(unassigned, key=lambda t: int(R_tw[t].sum()))
        members = [seed]
        q = R_tw[seed].copy()
        unassigned.remove(seed)
        while len(members) < NTMAX and unassigned:
            nt = len(members)
            best, bq, bw = None, None, None
            for t in unassigned:
                q2 = np.maximum(q, R_tw[t])
                waste = ((nt + 1) * q2.sum()
                         - (nt * q.sum() + R_tw[t].sum()))
                if (nt + 1) * q2.sum() > CAPR:
                    continue
                if best is None or waste < bw:
                    best, bq, bw = t, q2, waste
            if best is None or bw > 6:
                break
            members.append(best)
            q = bq
            unassigned.remove(best)
        chunks_old.append(members)
        q_list.append(q)
    Q = np.stack(q_list)  # [nchunk, NWIN]
    nch = len(chunks_old)

    # relabel tiles: chunk members become consecutive tile indices
    tile_perm = np.empty(TILES, np.int64)  # old tile -> new tile
    newt = 0
    chunks = []
    for ts in chunks_old:
        chunks.append(list(range(newt, newt + len(ts))))
        for t in ts:
            tile_perm[t] = newt
            newt += 1
    # move node positions: local (t, p) -> (tile_perm[t], p)
    loc = perm_pos % SHARD
    perm_pos = ((perm_pos // SHARD) * SHARD
                + tile_perm[loc // 128] * 128 + loc % 128)
    inv = np.full(FULL, -1, np.int64)
    inv[perm_pos] = np.arange(N)

    (src_p, dst_core, win_id, win_rel, tile_id, part_id,
     R_tw2) = edge_maps(perm_pos)
    assert (R_tw2[np.concatenate(chunks)] <= np.repeat(
        Q, [len(ts) for ts in chunks], axis=0)).all()

    col0w = np.zeros((nch, NWIN), np.int64)
    run = 0
    for s in range(nch):
        for w in range(NWIN):
            col0w[s, w] = run
            run += len(chunks[s]) * int(Q[s, w])
    TR = int(run)

    # per-edge global column
    chunk_of = np.zeros(TILES, np.int64)
    tl_of = np.zeros(TILES, np.int64)
    for s, ts in enumerate(chunks):
        for i, t in enumerate(ts):
            chunk_of[t] = s
            tl_of[t] = i
    # rank within (core, tile, part, window)
    key = (((dst_core * TILES + tile_id) * 128 + part_id) * NWIN + win_id)
    o = np.argsort(key, kind='stable')
    ks = key[o]
    starts = np.r_[0, np.flatnonzero(np.diff(ks)) + 1]
    group_len = np.diff(np.r_[starts, E])
    rank_sorted = np.arange(E) - np.repeat(starts, group_len)
    rank = np.empty(E, np.int64)
    rank[o] = rank_sorted

    s_of = chunk_of[tile_id]
    gcol = (col0w[s_of, win_id] + tl_of[tile_id] * Q[s_of, win_id] + rank)

    ew_all = []
    idxw_all = []
    for c in range(NCORES):
        m = dst_core == c
        ewf = np.zeros((128, TR), np.float32)
        ewf[part_id[m], gcol[m]] = ew[m]
        idx_flat = np.zeros(TR * 128, np.int16)
        idx_flat[gcol[m] * 128 + part_id[m]] = win_rel[m].astype(np.int16)
        iw = np.zeros((128, 8 * TR), np.int16)
        for s in range(nch):
            nt = len(chunks[s])
            for w in range(NWIN):
                rw_ = nt * int(Q[s, w])
                if rw_ == 0:
                    continue
                cb = int(col0w[s, w])
                seg = idx_flat[cb * 128:(cb + rw_) * 128]
                wseg = np.tile(seg.reshape(-1, 16).T, (8, 1))
                iw[:, 8 * cb:8 * (cb + rw_)] = wseg
        ew_all.append(ewf)
        idxw_all.append(iw)

    x_full = np.zeros((FULL, H), np.float32)
    x_full[perm_pos] = np.asarray(x, np.float32)
    x_own_all = [x_full[c * SHARD:(c + 1) * SHARD] for c in range(NCORES)]
    schedule = (chunks, Q, col0w, TR)
    return x_full, x_own_all, idxw_all, ew_all, schedule, inv


def _np_bf16(a):
    import ml_dtypes
    return np.asarray(a, np.float32).astype(ml_dtypes.bfloat16)


def kernel(x, edge_weight_norm, cheb_coeffs, group_scale, group_bias,
           rms_weight, edge_index):
    from concourse.bass_utils import run_bass_kernel_spmd

    x = np.asarray(x, np.float32)
    assert x.shape == (N, H)
    x_full, x_own_all, idxw_all, ew_all, schedule, inv = _prep(
        x, edge_weight_norm, edge_index)

    params = np.zeros((128, 40), np.float32)
    params[:, 32] = RMS_EPS
    cheb = np.asarray(cheb_coeffs, np.float32)      # [4, K+1]
    params[:, :24] = cheb.T.reshape(1, 24)          # k-major: col = 4k+g
    params[:, 24:28] = np.asarray(group_scale, np.float32).reshape(1, 4)
    params[:, 28:32] = np.asarray(group_bias, np.float32).reshape(1, 4)
    rmsw = np.tile(np.asarray(rms_weight, np.float32).reshape(1, H), (128, 1))

    repeat = int(os.environ.get("KERNEL_REPEAT", "1"))
    nc = _build(schedule, repeat=repeat)

    in_maps = []
    for c in range(NCORES):
        in_maps.append({
            "x_fullf": x_full,
            "x_own": x_own_all[c],
            "idx_in": idxw_all[c],
            "ew_in": ew_all[c],
            "par_in": params,
            "rw_in": rmsw,
        })
    res = run_bass_kernel_spmd(nc, in_maps, list(range(NCORES)))
    out_shards = np.stack([res.results[c]["out"] for c in range(NCORES)],
                          axis=0)
    out_full = out_shards.reshape(FULL, H)
    out = np.empty((N, H), np.float32)
    mask = inv >= 0
    out[inv[mask]] = out_full[mask]
    return out
